# revision 8
# baseline (speedup 1.0000x reference)
"""Adaptive embedding (Transformer-XL wt103) on 8 trn2 NeuronCores.

Strategy: token-parallel across the 8 cores (2048 tokens each, no
collectives). The host sorts each core's tokens by id so each embedding
bucket becomes one contiguous segment, dealt round-robin across cores
for near-perfect load balance.

Weight preprocessing (host, input-independent): buckets 0 and 1 have
d_emb 1024/256 with a dense projection to 1024 — the two linear maps
are folded into one pre-projected table pp01[v] = emb[v] @ projT *
sqrt(d_proj) in bf16. On device those tokens are a pure gather of 2KB
rows. Buckets 2/3 (d=64/16) stay factored: gathering their native-width
rows (128B/32B) plus a tiny projection load moves far fewer bytes than
pre-projected 2KB rows would.

Device graph per core: three batched indirect gathers (one per segment,
multi-column offset AP — one SWDGE call gathers all rows of a segment,
amortizing the ~1us per-call GpSimd descriptor-generation overhead),
then for buckets 2/3 a transpose (PE) -> psum copy -> matmul (K=d_emb)
-> bf16 output staging pipeline, and large contiguous output DMAs in a
partition-major layout. The output travels as bf16 (rel err ~4e-3, well
under the 2e-2 gate), halving the dominant output traffic. The host
undoes the sort permutation and widens to f32 on the way back.
"""

import os
import sys
import types

for _p in (
    "/root/.axon_site",
    "/root/.axon_site/_ro/trn_rl_repo",
    "/root/.axon_site/_ro/pypackages",
    "/opt/trn_rl_repo",
):
    if _p not in sys.path:
        sys.path.append(_p)

import numpy as np
import ml_dtypes

# antenv.axon_hooks shim: lets BASS_TRACE=1 profile runs work under axon.
try:
    import antenv.axon_hooks  # noqa: F401
except ImportError:
    _hooks = types.ModuleType("antenv.axon_hooks")
    _hooks._hook = None
    _hooks.set_axon_ntff_profile_hook = lambda h: setattr(_hooks, "_hook", h)
    _hooks.get_axon_ntff_profile_hook = lambda: _hooks._hook
    import antenv

    antenv.axon_hooks = _hooks
    sys.modules["antenv.axon_hooks"] = _hooks
    try:
        from trn_agent_boot.trn_boot import _ntff_profile_via_ctypes

        _h = _ntff_profile_via_ctypes("/opt/axon/libaxon_pjrt.so")
        if _h is not None:
            _hooks.set_axon_ntff_profile_hook(_h)
    except Exception:
        pass

import concourse.bacc as bacc
import concourse.bass as bass
import concourse.mybir as mybir
import concourse.tile as tile
from concourse.tile_rust import add_dep_helper
from concourse.bass_utils import run_bass_kernel_spmd

N_TOKEN = 267735
D_PROJ = 1024
EMB_SCALE = float(D_PROJ) ** 0.5
NCORES = 8
BF16 = ml_dtypes.bfloat16

# segments after folding buckets 0+1 into the pre-projected table:
# (global-id range, table row count, row width, kind)
SEGS = [
    {"name": "pp01", "lo": 0, "hi": 40000, "d": 1024, "kind": "direct"},
    {"name": "e2", "lo": 40000, "hi": 200000, "d": 64, "kind": "mm"},
    {"name": "e3", "lo": 200000, "hi": 267735, "d": 16, "kind": "mm"},
]
# gather order: e2 first so its compute pipeline starts as early as
# possible behind the serial SWDGE descriptor generation, then e3, then
# the compute-free pp01 pass-through whose tail is pure DMA
SEG_ORDER = [1, 2, 0]

LAST_RESULT = None  # BassKernelResults of the most recent run (for test.py)


def _build_graph(plan, nt_total, s_pad):
    """plan: per active segment (in gather order) a dict with
    si, nt, n_live, cb (idx col base), slot (output slot base)."""
    nc = bacc.Bacc(None, target_bir_lowering=False, debug=False)
    dt = mybir.dt

    tab_par = {}
    proj_par = {}
    for p in plan:
        s = SEGS[p["si"]]
        tab_par[p["si"]] = nc.declare_dram_parameter(
            s["name"], [s["hi"] - s["lo"], s["d"]], dt.bfloat16, False
        )
        if s["kind"] == "mm":
            proj_par[p["si"]] = nc.declare_dram_parameter(
                f"projt{p['si']}", [s["d"], D_PROJ], dt.bfloat16, False
            )
    ident_par = nc.declare_dram_parameter("ident", [128, 128], dt.bfloat16, False)
    idx_par = nc.declare_dram_parameter("idxs", [128, nt_total], dt.int32, False)
    # partition-major output: slot s lives at [s % 128, s // 128, :]
    out_par = nc.declare_dram_parameter(
        "out", [128, s_pad // 128, D_PROJ], dt.bfloat16, True
    )

    # raw (non-pool) SBUF tensors for the gather side: the tile framework
    # adds a semaphore-wait per tracked dependency on EVERY reader, which
    # costs ~250ns of GpSimd sequencer time per gather call. With raw
    # tensors only the FIRST gather carries an explicit dep on the idx
    # load; the rest follow via Pool-engine program order. Consumers get
    # explicit edges via add_dep_helper.
    idx_sb = nc.alloc_sbuf_tensor("idxsb", [128, nt_total], dt.int32)
    et_sb = {}
    for p in plan:
        si, nt = p["si"], p["nt"]
        d = SEGS[si]["d"]
        et_sb[si] = nc.alloc_sbuf_tensor(f"etsb{si}", [128, nt * d], dt.bfloat16)

    with tile.TileContext(nc) as tc:
        with (
            tc.tile_pool(name="const", bufs=1) as cpool,
            tc.tile_pool(name="lhsT", bufs=8) as ltpool,
            tc.tile_pool(name="outs", bufs=6) as opool,
            tc.tile_pool(name="ps", bufs=5, space="PSUM") as ppool,
            tc.tile_pool(name="ptr", bufs=3, space="PSUM") as trpool,
        ):
            # idx first on the sync HWDGE ring, then the small constants
            ixdma = nc.sync.dma_start(idx_sb[:], idx_par[:])
            ident = cpool.tile([128, 128], dt.bfloat16, tag="ident")
            nc.sync.dma_start(ident[:], ident_par[:])
            proj_sb = {}
            for p in plan:
                si = p["si"]
                if SEGS[si]["kind"] != "mm":
                    continue
                d = SEGS[si]["d"]
                pt = cpool.tile([d, D_PROJ], dt.bfloat16, tag=f"proj{si}")
                nc.sync.dma_start(pt[:], proj_par[si][:])
                proj_sb[si] = pt

            # indirect gathers, one per 128-token tile (the SWDGE ucode
            # generates one descriptor per partition: row idx_sb[p, col] of
            # the table lands in partition p; multi-column offset APs are NOT
            # supported by the hardware ucode). The ~1.1us/call descriptor
            # generation serializes on GpSimd and is this kernel's critical
            # path; everything else hides under it.
            gather_insts = {}
            first_gather = None
            for p in plan:
                si, nt = p["si"], p["nt"]
                d = SEGS[si]["d"]
                for t in range(nt):
                    gi = nc.gpsimd.indirect_dma_start(
                        out=et_sb[si][:, t * d : (t + 1) * d],
                        out_offset=None,
                        in_=tab_par[si][:],
                        in_offset=bass.IndirectOffsetOnAxis(
                            ap=idx_sb[:, p["cb"] + t : p["cb"] + t + 1], axis=0
                        ),
                    )
                    gather_insts[(si, t)] = gi
                    if first_gather is None:
                        first_gather = gi
                        add_dep_helper(
                            gi.ins, ixdma.ins, reason="first gather reads idxs"
                        )

            n_copies = 0
            n_lcopies = 0

            def _copy(dst, src):
                # alternate psum->sbuf copies across vector/scalar (GpSimd
                # cannot access PSUM)
                nonlocal n_copies
                if n_copies % 2 == 0:
                    nc.vector.tensor_copy(dst, src)
                else:
                    nc.scalar.copy(dst, src)
                n_copies += 1

            # matmul segments (e2 then e3); all outs ride the sync ring in
            # data-readiness order (SP does nothing else after the preamble)
            for p in plan:
                si, nt, n_live = p["si"], p["nt"], p["n_live"]
                seg = SEGS[si]
                if seg["kind"] != "mm":
                    continue
                d = seg["d"]
                nrow_last = (n_live - 1) % 128 + 1
                for tb in range(0, nt, 2):
                    gsz = min(2, nt - tb)
                    out_sb = opool.tile(
                        [128, 2, D_PROJ], dt.bfloat16, tag="osb", name="osb"
                    )
                    for ti in range(gsz):
                        t = tb + ti
                        ptr = trpool.tile([d, 128], dt.bfloat16, tag="ptr", name="ptr")
                        tr = nc.tensor.transpose(
                            ptr[:], et_sb[si][:, t * d : (t + 1) * d], ident[:]
                        )
                        add_dep_helper(
                            tr.ins, gather_insts[(si, t)].ins,
                            reason="transpose reads gathered tile",
                        )
                        lhsT = ltpool.tile([d, 128], dt.bfloat16, tag="lt", name="lt")
                        if n_lcopies % 2 == 0:
                            nc.vector.tensor_copy(lhsT[:], ptr[:])
                        else:
                            nc.scalar.copy(lhsT[:], ptr[:])
                        n_lcopies += 1
                        for nh in range(2):
                            ps = ppool.tile([128, 512], dt.float32, tag="ps", name="ps")
                            nc.tensor.matmul(
                                ps[:],
                                lhsT[:],
                                proj_sb[si][:, nh * 512 : (nh + 1) * 512],
                                start=True,
                                stop=True,
                            )
                            _copy(out_sb[:, ti, nh * 512 : (nh + 1) * 512], ps[:])
                    t0 = p["slot"] // 128 + tb
                    has_partial = (tb + gsz == nt) and nrow_last < 128
                    nfull = gsz - 1 if has_partial else gsz
                    if nfull:
                        nc.sync.dma_start(
                            out_par[:, t0 : t0 + nfull, :], out_sb[:, :nfull, :]
                        )
                    if has_partial:
                        nc.sync.dma_start(
                            out_par[:nrow_last, t0 + nfull, :],
                            out_sb[:nrow_last, nfull, :],
                        )

            # direct (pre-projected) segment last: gathered rows ARE output
            # rows, pure DMA pass-through as the tail
            for p in plan:
                si, nt, n_live = p["si"], p["nt"], p["n_live"]
                if SEGS[si]["kind"] != "direct":
                    continue
                nrow_last = (n_live - 1) % 128 + 1
                t0 = p["slot"] // 128
                for t in range(nt):
                    nrow = nrow_last if t == nt - 1 else 128
                    od = nc.sync.dma_start(
                        out_par[:nrow, t0 + t, :],
                        et_sb[si][:nrow, t * D_PROJ : (t + 1) * D_PROJ],
                    )
                    add_dep_helper(
                        od.ins, gather_insts[(si, t)].ins,
                        reason="out reads gathered rows",
                    )

    nc.compile()
    return nc


def _host_tables(emb0, emb1, emb2, emb3, proj0, proj1, proj2, proj3):
    # fold embedding + projection of buckets 0/1 into one table (f32
    # accumulate, then bf16)
    pp0 = (emb0 @ proj0.T) * EMB_SCALE
    pp1 = (emb1 @ proj1.T) * EMB_SCALE
    pp01 = np.ascontiguousarray(
        np.concatenate([pp0, pp1], axis=0).astype(BF16)
    )
    e2 = np.ascontiguousarray(emb2.astype(BF16))
    e3 = np.ascontiguousarray(emb3.astype(BF16))
    p2t = np.ascontiguousarray((proj2.T * EMB_SCALE).astype(BF16))
    p3t = np.ascontiguousarray((proj3.T * EMB_SCALE).astype(BF16))
    return pp01, e2, e3, p2t, p3t


def kernel(inp, emb0, emb1, emb2, emb3, proj0, proj1, proj2, proj3):
    global LAST_RESULT
    ids = np.asarray(inp).reshape(-1).astype(np.int64)
    n_tok = ids.shape[0]
    assert n_tok % NCORES == 0

    pp01, e2, e3, p2t, p3t = _host_tables(
        np.asarray(emb0), np.asarray(emb1), np.asarray(emb2), np.asarray(emb3),
        np.asarray(proj0), np.asarray(proj1), np.asarray(proj2), np.asarray(proj3),
    )
    tables = {"pp01": pp01, "e2": e2, "e3": e3}
    ident_host = np.eye(128, dtype=BF16)

    # --- sort + segment + deal round-robin to cores ---
    order = np.argsort(ids, kind="stable")
    sids = ids[order]

    plan = []  # per active segment: si, nt, n_live, cb, slot
    core_idx = [[] for _ in range(NCORES)]
    unshard = []  # (slot_base, [global token positions per core])
    cb = 0
    slot = 0
    for si in SEG_ORDER:
        seg = SEGS[si]
        g_lo = np.searchsorted(sids, seg["lo"], "left")
        g_hi = np.searchsorted(sids, seg["hi"], "left")
        if g_hi <= g_lo:
            continue
        toks = order[g_lo:g_hi]
        locs = (sids[g_lo:g_hi] - seg["lo"]).astype(np.int32)
        counts = [len(locs[c::NCORES]) for c in range(NCORES)]
        n_live = max(counts)
        n_pad = -(-n_live // 128) * 128
        nt = n_pad // 128
        per_core_toks = []
        for c in range(NCORES):
            li = locs[c::NCORES]
            pad = np.zeros(n_pad, np.int32)
            pad[: len(li)] = li
            # idx col cb+t, partition p holds the row for slot t*128+p
            core_idx[c].append(pad.reshape(nt, 128).T)
            per_core_toks.append(toks[c::NCORES])
        plan.append({"si": si, "nt": nt, "n_live": n_live, "cb": cb, "slot": slot})
        unshard.append((slot, per_core_toks))
        cb += nt
        slot += n_pad
    nt_total = cb
    s_pad = slot

    in_maps = []
    for c in range(NCORES):
        idx_host = np.ascontiguousarray(np.concatenate(core_idx[c], axis=1))
        m = {"ident": ident_host, "idxs": idx_host}
        for p in plan:
            s = SEGS[p["si"]]
            m[s["name"]] = tables[s["name"]]
            if s["kind"] == "mm":
                m[f"projt{p['si']}"] = p2t if p["si"] == 1 else p3t
        in_maps.append(m)

    nc = _build_graph(plan, nt_total, s_pad)
    res = run_bass_kernel_spmd(nc, in_maps, core_ids=list(range(NCORES)))
    LAST_RESULT = res

    # --- unshard: undo the sort permutation, widen to f32 ---
    full = np.empty((n_tok, D_PROJ), np.float32)
    for c in range(NCORES):
        oc = res.results[c]["out"]  # [128, T, 1024] bf16
        oc_rows = oc.transpose(1, 0, 2).reshape(-1, D_PROJ)  # slot-major
        for (slot0, per_core_toks) in unshard:
            toks = per_core_toks[c]
            if len(toks):
                full[toks] = oc_rows[slot0 : slot0 + len(toks)]
    B, S = np.asarray(inp).shape
    return full.reshape(B, S, D_PROJ)


# revision 9
# speedup vs baseline: 1.0105x; 1.0105x over previous
"""Adaptive embedding (Transformer-XL wt103) on 8 trn2 NeuronCores.

Strategy: token-parallel across the 8 cores (2048 tokens each, no
collectives). The host sorts each core's tokens by id so each embedding
bucket becomes one contiguous segment, dealt round-robin across cores
for near-perfect load balance.

Weight preprocessing (host, input-independent):
 - Buckets 0 and 1 (d_emb 1024/256, dense projection to 1024) are folded
   into one pre-projected table pp01[v] = emb[v] @ projT * sqrt(d_proj)
   in bf16. On device those tokens are a pure gather -> output DMA.
 - Buckets 2 and 3 (d=64/16) are merged into one 80-column table: e2
   rows occupy cols 0:64, e3 rows cols 64:80 (zero-padded elsewhere),
   stacked so local id = global id - 40000 for both. One stacked
   projection [80, 1024] = [proj2T; proj3T] makes every merged token the
   SAME matmul (the zero blocks kill the cross terms). This costs one
   fused gather+matmul pipeline instead of two and one fewer 128-token
   gather call.

Device graph per core: per 128-token tile, an offset-driven indirect
DMA gather (one row per partition; the SWDGE ucode supports exactly 128
descriptors per call, ~1.4us of serial GpSimd time each — the critical
path). Merged-bucket tiles run transpose (PE) -> psum copy -> matmul
(K=80) -> bf16 output staging; pp01 tiles ship straight out in a single
batched output DMA. Output travels as bf16 (rel err ~3e-3, well under
the 2e-2 gate), halving the dominant output traffic. The host undoes
the sort permutation and widens to f32 on the way back.
"""

import os
import sys
import types

for _p in (
    "/root/.axon_site",
    "/root/.axon_site/_ro/trn_rl_repo",
    "/root/.axon_site/_ro/pypackages",
    "/opt/trn_rl_repo",
):
    if _p not in sys.path:
        sys.path.append(_p)

import numpy as np
import ml_dtypes

# antenv.axon_hooks shim: lets BASS_TRACE=1 profile runs work under axon.
try:
    import antenv.axon_hooks  # noqa: F401
except ImportError:
    _hooks = types.ModuleType("antenv.axon_hooks")
    _hooks._hook = None
    _hooks.set_axon_ntff_profile_hook = lambda h: setattr(_hooks, "_hook", h)
    _hooks.get_axon_ntff_profile_hook = lambda: _hooks._hook
    import antenv

    antenv.axon_hooks = _hooks
    sys.modules["antenv.axon_hooks"] = _hooks
    try:
        from trn_agent_boot.trn_boot import _ntff_profile_via_ctypes

        _h = _ntff_profile_via_ctypes("/opt/axon/libaxon_pjrt.so")
        if _h is not None:
            _hooks.set_axon_ntff_profile_hook(_h)
    except Exception:
        pass

import concourse.bacc as bacc
import concourse.bass as bass
import concourse.mybir as mybir
import concourse.tile as tile
from concourse.tile_rust import add_dep_helper
from concourse.bass_utils import run_bass_kernel_spmd

N_TOKEN = 267735
D_PROJ = 1024
EMB_SCALE = float(D_PROJ) ** 0.5
NCORES = 8
BF16 = ml_dtypes.bfloat16
D23 = 80  # merged bucket-2/3 row width (64 + 16)

# two segments: pre-projected buckets 0+1 (direct), merged buckets 2+3
SEGS = [
    {"name": "pp01", "lo": 0, "hi": 40000, "d": D_PROJ, "kind": "direct"},
    {"name": "em23", "lo": 40000, "hi": N_TOKEN, "d": D23, "kind": "mm"},
]
# gather order: merged mm segment first so its compute pipeline starts
# as early as possible behind the serial SWDGE descriptor generation;
# the compute-free pp01 pass-through last (pure-DMA tail)
SEG_ORDER = [1, 0]

LAST_RESULT = None  # BassKernelResults of the most recent run (for test.py)


def _build_graph(plan, nt_total, s_pad):
    """plan: per active segment (in gather order) a dict with
    si, nt, n_live, cb (idx col base), slot (output slot base)."""
    nc = bacc.Bacc(None, target_bir_lowering=False, debug=False)
    dt = mybir.dt

    tab_par = {}
    for p in plan:
        s = SEGS[p["si"]]
        tab_par[p["si"]] = nc.declare_dram_parameter(
            s["name"], [s["hi"] - s["lo"], s["d"]], dt.bfloat16, False
        )
    proj_par = nc.declare_dram_parameter("projt", [D23, D_PROJ], dt.bfloat16, False)
    ident_par = nc.declare_dram_parameter("ident", [128, 128], dt.bfloat16, False)
    idx_par = nc.declare_dram_parameter("idxs", [128, nt_total], dt.int32, False)
    # partition-major output: slot s lives at [s % 128, s // 128, :]
    out_par = nc.declare_dram_parameter(
        "out", [128, s_pad // 128, D_PROJ], dt.bfloat16, True
    )

    # raw (non-pool) SBUF tensors for the gather side: only the FIRST
    # gather carries an explicit dep on the idx load; the rest follow via
    # Pool-engine program order. Consumers get explicit add_dep_helper
    # edges (the tile framework does not track raw tensors).
    idx_sb = nc.alloc_sbuf_tensor("idxsb", [128, nt_total], dt.int32)
    et_sb = {}
    for p in plan:
        si, nt = p["si"], p["nt"]
        d = SEGS[si]["d"]
        et_sb[si] = nc.alloc_sbuf_tensor(f"etsb{si}", [128, nt * d], dt.bfloat16)

    with tile.TileContext(nc) as tc:
        with (
            tc.tile_pool(name="const", bufs=1) as cpool,
            tc.tile_pool(name="lhsT", bufs=8) as ltpool,
            tc.tile_pool(name="outs", bufs=6) as opool,
            tc.tile_pool(name="ps", bufs=5, space="PSUM") as ppool,
            tc.tile_pool(name="ptr", bufs=3, space="PSUM") as trpool,
        ):
            # idx first on the sync HWDGE ring, then the small constants
            ixdma = nc.sync.dma_start(idx_sb[:], idx_par[:])
            ident = cpool.tile([128, 128], dt.bfloat16, tag="ident")
            nc.sync.dma_start(ident[:], ident_par[:])
            proj_sb = cpool.tile([D23, D_PROJ], dt.bfloat16, tag="proj")
            nc.sync.dma_start(proj_sb[:], proj_par[:])

            # indirect gathers, one per 128-token tile (the SWDGE ucode
            # generates one descriptor per partition: row idx_sb[p, col] of
            # the table lands in partition p; multi-column offset APs are NOT
            # supported by the hardware ucode). The ~1.4us/call serial GpSimd
            # time is the kernel's critical path.
            gather_insts = {}
            first_gather = None
            for p in plan:
                si, nt = p["si"], p["nt"]
                d = SEGS[si]["d"]
                for t in range(nt):
                    gi = nc.gpsimd.indirect_dma_start(
                        out=et_sb[si][:, t * d : (t + 1) * d],
                        out_offset=None,
                        in_=tab_par[si][:],
                        in_offset=bass.IndirectOffsetOnAxis(
                            ap=idx_sb[:, p["cb"] + t : p["cb"] + t + 1], axis=0
                        ),
                    )
                    gather_insts[(si, t)] = gi
                    if first_gather is None:
                        first_gather = gi
                        add_dep_helper(
                            gi.ins, ixdma.ins, reason="first gather reads idxs"
                        )

            n_copies = 0
            n_lcopies = 0

            def _copy(dst, src):
                # alternate psum->sbuf copies across vector/scalar (GpSimd
                # cannot access PSUM)
                nonlocal n_copies
                if n_copies % 2 == 0:
                    nc.vector.tensor_copy(dst, src)
                else:
                    nc.scalar.copy(dst, src)
                n_copies += 1

            # merged mm segment; all outs ride the sync ring in
            # data-readiness order (SP does nothing else after the preamble)
            for p in plan:
                si, nt, n_live = p["si"], p["nt"], p["n_live"]
                seg = SEGS[si]
                if seg["kind"] != "mm":
                    continue
                d = seg["d"]
                nrow_last = (n_live - 1) % 128 + 1
                for tb in range(0, nt, 2):
                    gsz = min(2, nt - tb)
                    out_sb = opool.tile(
                        [128, 2, D_PROJ], dt.bfloat16, tag="osb", name="osb"
                    )
                    for ti in range(gsz):
                        t = tb + ti
                        ptr = trpool.tile([d, 128], dt.bfloat16, tag="ptr", name="ptr")
                        tr = nc.tensor.transpose(
                            ptr[:], et_sb[si][:, t * d : (t + 1) * d], ident[:]
                        )
                        add_dep_helper(
                            tr.ins, gather_insts[(si, t)].ins,
                            reason="transpose reads gathered tile",
                        )
                        lhsT = ltpool.tile([d, 128], dt.bfloat16, tag="lt", name="lt")
                        if n_lcopies % 2 == 0:
                            nc.vector.tensor_copy(lhsT[:], ptr[:])
                        else:
                            nc.scalar.copy(lhsT[:], ptr[:])
                        n_lcopies += 1
                        for nh in range(2):
                            ps = ppool.tile([128, 512], dt.float32, tag="ps", name="ps")
                            nc.tensor.matmul(
                                ps[:],
                                lhsT[:],
                                proj_sb[:, nh * 512 : (nh + 1) * 512],
                                start=True,
                                stop=True,
                            )
                            _copy(out_sb[:, ti, nh * 512 : (nh + 1) * 512], ps[:])
                    t0 = p["slot"] // 128 + tb
                    has_partial = (tb + gsz == nt) and nrow_last < 128
                    nfull = gsz - 1 if has_partial else gsz
                    if nfull:
                        nc.sync.dma_start(
                            out_par[:, t0 : t0 + nfull, :], out_sb[:, :nfull, :]
                        )
                    if has_partial:
                        nc.sync.dma_start(
                            out_par[:nrow_last, t0 + nfull, :],
                            out_sb[:nrow_last, nfull, :],
                        )

            # direct (pre-projected) segment: gathered rows ARE output rows.
            # One batched full-tile DMA (garbage in padding slots is fine —
            # the host only reads live rows); a single wide DMA spreads its
            # descriptors across all 16 queues instead of crawling on one.
            for p in plan:
                si, nt, n_live = p["si"], p["nt"], p["n_live"]
                if SEGS[si]["kind"] != "direct":
                    continue
                t0 = p["slot"] // 128
                od = nc.sync.dma_start(
                    out_par[:, t0 : t0 + nt, :], et_sb[si][:, : nt * D_PROJ]
                )
                for t in range(nt):
                    add_dep_helper(
                        od.ins, gather_insts[(si, t)].ins,
                        reason="out reads gathered rows",
                    )

    nc.compile()
    return nc


def _host_tables(emb0, emb1, emb2, emb3, proj0, proj1, proj2, proj3):
    # fold embedding + projection of buckets 0/1 into one table (f32
    # accumulate, then bf16)
    pp0 = (emb0 @ proj0.T) * EMB_SCALE
    pp1 = (emb1 @ proj1.T) * EMB_SCALE
    pp01 = np.ascontiguousarray(
        np.concatenate([pp0, pp1], axis=0).astype(BF16)
    )
    # merged bucket-2/3 table: e2 rows in cols 0:64, e3 rows in cols 64:80
    em23 = np.zeros((N_TOKEN - 40000, D23), BF16)
    em23[: emb2.shape[0], :64] = emb2.astype(BF16)
    em23[emb2.shape[0] :, 64:] = emb3.astype(BF16)
    # stacked projection [80, 1024]
    p23 = np.zeros((D23, D_PROJ), np.float32)
    p23[:64] = proj2.T * EMB_SCALE
    p23[64:] = proj3.T * EMB_SCALE
    return pp01, np.ascontiguousarray(em23), np.ascontiguousarray(p23.astype(BF16))


def kernel(inp, emb0, emb1, emb2, emb3, proj0, proj1, proj2, proj3):
    global LAST_RESULT
    ids = np.asarray(inp).reshape(-1).astype(np.int64)
    n_tok = ids.shape[0]
    assert n_tok % NCORES == 0

    pp01, em23, p23 = _host_tables(
        np.asarray(emb0), np.asarray(emb1), np.asarray(emb2), np.asarray(emb3),
        np.asarray(proj0), np.asarray(proj1), np.asarray(proj2), np.asarray(proj3),
    )
    tables = {"pp01": pp01, "em23": em23}
    ident_host = np.eye(128, dtype=BF16)

    # --- sort + segment + deal round-robin to cores ---
    order = np.argsort(ids, kind="stable")
    sids = ids[order]

    plan = []  # per active segment: si, nt, n_live, cb, slot
    core_idx = [[] for _ in range(NCORES)]
    unshard = []  # (slot_base, [global token positions per core])
    cb = 0
    slot = 0
    for si in SEG_ORDER:
        seg = SEGS[si]
        g_lo = np.searchsorted(sids, seg["lo"], "left")
        g_hi = np.searchsorted(sids, seg["hi"], "left")
        if g_hi <= g_lo:
            continue
        toks = order[g_lo:g_hi]
        locs = (sids[g_lo:g_hi] - seg["lo"]).astype(np.int32)
        counts = [len(locs[c::NCORES]) for c in range(NCORES)]
        n_live = max(counts)
        n_pad = -(-n_live // 128) * 128
        nt = n_pad // 128
        per_core_toks = []
        for c in range(NCORES):
            li = locs[c::NCORES]
            pad = np.zeros(n_pad, np.int32)
            pad[: len(li)] = li
            # idx col cb+t, partition p holds the row for slot t*128+p
            core_idx[c].append(pad.reshape(nt, 128).T)
            per_core_toks.append(toks[c::NCORES])
        plan.append({"si": si, "nt": nt, "n_live": n_live, "cb": cb, "slot": slot})
        unshard.append((slot, per_core_toks))
        cb += nt
        slot += n_pad
    nt_total = cb
    s_pad = slot

    in_maps = []
    for c in range(NCORES):
        idx_host = np.ascontiguousarray(np.concatenate(core_idx[c], axis=1))
        m = {"ident": ident_host, "idxs": idx_host, "projt": p23}
        for p in plan:
            s = SEGS[p["si"]]
            m[s["name"]] = tables[s["name"]]
        in_maps.append(m)

    nc = _build_graph(plan, nt_total, s_pad)
    res = run_bass_kernel_spmd(nc, in_maps, core_ids=list(range(NCORES)))
    LAST_RESULT = res

    # --- unshard: undo the sort permutation, widen to f32 ---
    full = np.empty((n_tok, D_PROJ), np.float32)
    for c in range(NCORES):
        oc = res.results[c]["out"]  # [128, T, 1024] bf16
        oc_rows = oc.transpose(1, 0, 2).reshape(-1, D_PROJ)  # slot-major
        for (slot0, per_core_toks) in unshard:
            toks = per_core_toks[c]
            if len(toks):
                full[toks] = oc_rows[slot0 : slot0 + len(toks)]
    B, S = np.asarray(inp).shape
    return full.reshape(B, S, D_PROJ)


# revision 20
# speedup vs baseline: 1.0235x; 1.0128x over previous
"""Adaptive embedding (Transformer-XL wt103) on 8 trn2 NeuronCores.

Strategy: token-parallel across the 8 cores (2048 tokens each, no
collectives). The host sorts each core's tokens by id so each embedding
bucket becomes one contiguous segment, dealt round-robin across cores
for near-perfect load balance.

Weight preprocessing (host, input-independent):
 - Buckets 0 and 1 (d_emb 1024/256, dense projection to 1024) are folded
   into one pre-projected table pp01[v] = emb[v] @ projT * sqrt(d_proj)
   in bf16. On device those tokens are a pure gather -> output DMA.
 - Buckets 2 and 3 (d=64/16) are merged into one 80-column table: e2
   rows occupy cols 0:64, e3 rows cols 64:80 (zero-padded elsewhere),
   stacked so local id = global id - 40000 for both. One stacked
   projection [80, 1024] = [proj2T; proj3T] makes every merged token the
   SAME matmul (the zero blocks kill the cross terms). This costs one
   fused gather+matmul pipeline instead of two and one fewer 128-token
   gather call.

Device graph per core: per 128-token tile, an offset-driven indirect
DMA gather (one row per partition; the SWDGE ucode supports exactly 128
descriptors per call, ~1.4us of serial GpSimd time each — the critical
path). Merged-bucket tiles run transpose (PE) -> psum copy -> matmul
(K=80) -> bf16 output staging; pp01 tiles ship straight out in a single
batched output DMA. Output travels as bf16 (rel err ~3e-3, well under
the 2e-2 gate), halving the dominant output traffic. The host undoes
the sort permutation and widens to f32 on the way back.
"""

import os
import sys
import types

for _p in (
    "/root/.axon_site",
    "/root/.axon_site/_ro/trn_rl_repo",
    "/root/.axon_site/_ro/pypackages",
    "/opt/trn_rl_repo",
):
    if _p not in sys.path:
        sys.path.append(_p)

import numpy as np
import ml_dtypes

# antenv.axon_hooks shim: lets BASS_TRACE=1 profile runs work under axon.
try:
    import antenv.axon_hooks  # noqa: F401
except ImportError:
    _hooks = types.ModuleType("antenv.axon_hooks")
    _hooks._hook = None
    _hooks.set_axon_ntff_profile_hook = lambda h: setattr(_hooks, "_hook", h)
    _hooks.get_axon_ntff_profile_hook = lambda: _hooks._hook
    import antenv

    antenv.axon_hooks = _hooks
    sys.modules["antenv.axon_hooks"] = _hooks
    try:
        from trn_agent_boot.trn_boot import _ntff_profile_via_ctypes

        _h = _ntff_profile_via_ctypes("/opt/axon/libaxon_pjrt.so")
        if _h is not None:
            _hooks.set_axon_ntff_profile_hook(_h)
    except Exception:
        pass

import concourse.bacc as bacc
import concourse.bass as bass
import concourse.mybir as mybir
import concourse.tile as tile
from concourse.tile_rust import add_dep_helper
from concourse.bass_utils import run_bass_kernel_spmd

N_TOKEN = 267735
D_PROJ = 1024
EMB_SCALE = float(D_PROJ) ** 0.5
NCORES = 8
BF16 = ml_dtypes.bfloat16
D23 = 80  # merged bucket-2/3 row width (64 + 16)

# two segments: pre-projected buckets 0+1 (direct), merged buckets 2+3
SEGS = [
    {"name": "pp01", "lo": 0, "hi": 40000, "d": D_PROJ, "kind": "direct"},
    {"name": "em23", "lo": 40000, "hi": N_TOKEN, "d": D23, "kind": "mm"},
]
# gather order: merged mm segment first so its compute pipeline starts
# as early as possible behind the serial SWDGE descriptor generation;
# the compute-free pp01 pass-through last (pure-DMA tail)
SEG_ORDER = [1, 0]

LAST_RESULT = None  # BassKernelResults of the most recent run (for test.py)


def _build_graph(plan, nt_total, s_pad):
    """plan: per active segment (in gather order) a dict with
    si, nt, n_live, cb (idx col base), slot (output slot base)."""
    nc = bacc.Bacc(None, target_bir_lowering=False, debug=False)
    dt = mybir.dt

    tab_par = {}
    for p in plan:
        s = SEGS[p["si"]]
        tab_par[p["si"]] = nc.declare_dram_parameter(
            s["name"], [s["hi"] - s["lo"], s["d"]], dt.bfloat16, False
        )
    proj_par = nc.declare_dram_parameter("projt", [D23, D_PROJ], dt.bfloat16, False)
    ident_par = nc.declare_dram_parameter("ident", [128, 128], dt.bfloat16, False)
    idx_par = nc.declare_dram_parameter("idxs", [128, nt_total], dt.int32, False)
    # partition-major outputs: slot s lives at [s % 128, s // 128, :].
    # Matmul-segment tiles ship straight from PSUM as f32 (no cast stage);
    # pp01 tiles pass through as bf16.
    nt_mm = sum(p["nt"] for p in plan if SEGS[p["si"]]["kind"] == "mm")
    nt_pp = sum(p["nt"] for p in plan if SEGS[p["si"]]["kind"] == "direct")
    outmm_par = nc.declare_dram_parameter(
        "outmm", [128, max(nt_mm, 1), D_PROJ], dt.bfloat16, True
    )
    outpp_par = nc.declare_dram_parameter(
        "outpp", [128, max(nt_pp, 1), D_PROJ], dt.bfloat16, True
    )

    # raw (non-pool) SBUF tensors for the gather side: only the FIRST
    # gather carries an explicit dep on the idx load; the rest follow via
    # Pool-engine program order. Consumers get explicit add_dep_helper
    # edges (the tile framework does not track raw tensors).
    idx_sb = nc.alloc_sbuf_tensor("idxsb", [128, nt_total], dt.int32)
    et_sb = {}
    for p in plan:
        si, nt = p["si"], p["nt"]
        d = SEGS[si]["d"]
        et_sb[si] = nc.alloc_sbuf_tensor(f"etsb{si}", [128, nt * d], dt.bfloat16)

    with tile.TileContext(nc) as tc:
        with (
            tc.tile_pool(name="const", bufs=1) as cpool,
            tc.tile_pool(name="lhsT", bufs=8) as ltpool,
            tc.tile_pool(name="outs", bufs=6) as opool,
            tc.tile_pool(name="ps", bufs=5, space="PSUM") as ppool,
            tc.tile_pool(name="ptr", bufs=3, space="PSUM") as trpool,
        ):
            # idx first on the sync HWDGE ring, then the small constants
            ixdma = nc.sync.dma_start(idx_sb[:], idx_par[:])
            ident = cpool.tile([128, 128], dt.bfloat16, tag="ident")
            nc.sync.dma_start(ident[:], ident_par[:])
            proj_sb = cpool.tile([D23, D_PROJ], dt.bfloat16, tag="proj")
            nc.sync.dma_start(proj_sb[:], proj_par[:])

            # indirect gathers, one per 128-token tile (the SWDGE ucode
            # generates one descriptor per partition: row idx_sb[p, col] of
            # the table lands in partition p; multi-column offset APs are NOT
            # supported by the hardware ucode, and offsets must live in SBUF).
            # The ~1.4us/call serial GpSimd time is the kernel's critical path.
            gather_insts = {}
            first_gather = None
            for p in plan:
                si, nt = p["si"], p["nt"]
                d = SEGS[si]["d"]
                for t in range(nt):
                    gi = nc.gpsimd.indirect_dma_start(
                        out=et_sb[si][:, t * d : (t + 1) * d],
                        out_offset=None,
                        in_=tab_par[si][:],
                        in_offset=bass.IndirectOffsetOnAxis(
                            ap=idx_sb[:, p["cb"] + t : p["cb"] + t + 1], axis=0
                        ),
                    )
                    gather_insts[(si, t)] = gi
                    if first_gather is None:
                        first_gather = gi
                        add_dep_helper(
                            gi.ins, ixdma.ins, reason="first gather reads idxs"
                        )

            # merged mm segment: per tile, transpose (PE) -> lhsT copy ->
            # two matmuls -> two psum->bf16 casts -> batched out DMA on the
            # sync ring (SP does nothing else after the preamble). Engine
            # schedule avoids ping-pong bubbles: tile t's BOTH casts run on
            # engine t%2 while its lhsT copy runs on the OTHER engine.
            engs = [nc.vector, nc.scalar]
            for p in plan:
                si, nt, n_live = p["si"], p["nt"], p["n_live"]
                seg = SEGS[si]
                if seg["kind"] != "mm":
                    continue
                d = seg["d"]
                nrow_last = (n_live - 1) % 128 + 1
                for tb in range(0, nt, 2):
                    gsz = min(2, nt - tb)
                    out_sb = opool.tile(
                        [128, 2, D_PROJ], dt.bfloat16, tag="osb", name="osb"
                    )
                    for ti in range(gsz):
                        t = tb + ti
                        ptr = trpool.tile([d, 128], dt.bfloat16, tag="ptr", name="ptr")
                        tr = nc.tensor.transpose(
                            ptr[:], et_sb[si][:, t * d : (t + 1) * d], ident[:]
                        )
                        add_dep_helper(
                            tr.ins, gather_insts[(si, t)].ins,
                            reason="transpose reads gathered tile",
                        )
                        lhsT = ltpool.tile([d, 128], dt.bfloat16, tag="lt", name="lt")
                        ceng = engs[t % 2]
                        oeng = engs[(t + 1) % 2]
                        if oeng is nc.vector:
                            oeng.tensor_copy(lhsT[:], ptr[:])
                        else:
                            oeng.copy(lhsT[:], ptr[:])
                        for nh in range(2):
                            ps = ppool.tile([128, 512], dt.float32, tag="ps", name="ps")
                            nc.tensor.matmul(
                                ps[:],
                                lhsT[:],
                                proj_sb[:, nh * 512 : (nh + 1) * 512],
                                start=True,
                                stop=True,
                            )
                            dst = out_sb[:, ti, nh * 512 : (nh + 1) * 512]
                            if ceng is nc.vector:
                                ceng.tensor_copy(dst, ps[:])
                            else:
                                ceng.copy(dst, ps[:])
                    t0 = p["obase"] + tb
                    has_partial = (tb + gsz == nt) and nrow_last < 128
                    nfull = gsz - 1 if has_partial else gsz
                    if nfull:
                        nc.sync.dma_start(
                            outmm_par[:, t0 : t0 + nfull, :], out_sb[:, :nfull, :]
                        )
                    if has_partial:
                        nc.sync.dma_start(
                            outmm_par[:nrow_last, t0 + nfull, :],
                            out_sb[:nrow_last, nfull, :],
                        )

            # direct (pre-projected) segment: gathered rows ARE output rows.
            # One batched full-tile DMA from the idle GpSimd ring (garbage in
            # padding slots is fine — the host only reads live rows).
            for p in plan:
                si, nt, n_live = p["si"], p["nt"], p["n_live"]
                if SEGS[si]["kind"] != "direct":
                    continue
                t0 = p["obase"]
                od = nc.gpsimd.dma_start(
                    outpp_par[:, t0 : t0 + nt, :], et_sb[si][:, : nt * D_PROJ]
                )
                for t in range(nt):
                    add_dep_helper(
                        od.ins, gather_insts[(si, t)].ins,
                        reason="out reads gathered rows",
                    )

    nc.compile()
    return nc


def _host_tables(emb0, emb1, emb2, emb3, proj0, proj1, proj2, proj3):
    # fold embedding + projection of buckets 0/1 into one table (f32
    # accumulate, then bf16)
    pp0 = (emb0 @ proj0.T) * EMB_SCALE
    pp1 = (emb1 @ proj1.T) * EMB_SCALE
    pp01 = np.ascontiguousarray(
        np.concatenate([pp0, pp1], axis=0).astype(BF16)
    )
    # merged bucket-2/3 table: e2 rows in cols 0:64, e3 rows in cols 64:80
    em23 = np.zeros((N_TOKEN - 40000, D23), BF16)
    em23[: emb2.shape[0], :64] = emb2.astype(BF16)
    em23[emb2.shape[0] :, 64:] = emb3.astype(BF16)
    # stacked projection [80, 1024]
    p23 = np.zeros((D23, D_PROJ), np.float32)
    p23[:64] = proj2.T * EMB_SCALE
    p23[64:] = proj3.T * EMB_SCALE
    return pp01, np.ascontiguousarray(em23), np.ascontiguousarray(p23.astype(BF16))


def kernel(inp, emb0, emb1, emb2, emb3, proj0, proj1, proj2, proj3):
    global LAST_RESULT
    ids = np.asarray(inp).reshape(-1).astype(np.int64)
    n_tok = ids.shape[0]
    assert n_tok % NCORES == 0

    pp01, em23, p23 = _host_tables(
        np.asarray(emb0), np.asarray(emb1), np.asarray(emb2), np.asarray(emb3),
        np.asarray(proj0), np.asarray(proj1), np.asarray(proj2), np.asarray(proj3),
    )
    tables = {"pp01": pp01, "em23": em23}
    ident_host = np.eye(128, dtype=BF16)

    # --- sort + segment + deal round-robin to cores ---
    order = np.argsort(ids, kind="stable")
    sids = ids[order]

    plan = []  # per active segment: si, nt, n_live, cb, obase
    core_idx = [[] for _ in range(NCORES)]
    unshard = []  # (out tensor name, tile base, [token positions per core])
    cb = 0
    obase = {"mm": 0, "direct": 0}
    for si in SEG_ORDER:
        seg = SEGS[si]
        g_lo = np.searchsorted(sids, seg["lo"], "left")
        g_hi = np.searchsorted(sids, seg["hi"], "left")
        if g_hi <= g_lo:
            continue
        toks = order[g_lo:g_hi]
        locs = (sids[g_lo:g_hi] - seg["lo"]).astype(np.int32)
        counts = [len(locs[c::NCORES]) for c in range(NCORES)]
        n_live = max(counts)
        n_pad = -(-n_live // 128) * 128
        nt = n_pad // 128
        per_core_toks = []
        for c in range(NCORES):
            li = locs[c::NCORES]
            pad = np.zeros(n_pad, np.int32)
            pad[: len(li)] = li
            # idx col cb+t, partition p holds the row for slot t*128+p
            core_idx[c].append(pad.reshape(nt, 128).T)
            per_core_toks.append(toks[c::NCORES])
        kind = seg["kind"]
        plan.append(
            {"si": si, "nt": nt, "n_live": n_live, "cb": cb, "obase": obase[kind]}
        )
        unshard.append(
            ("outmm" if kind == "mm" else "outpp", obase[kind], per_core_toks)
        )
        cb += nt
        obase[kind] += nt
    nt_total = cb

    in_maps = []
    for c in range(NCORES):
        idx_host = np.ascontiguousarray(np.concatenate(core_idx[c], axis=1))
        m = {"ident": ident_host, "idxs": idx_host, "projt": p23}
        for p in plan:
            s = SEGS[p["si"]]
            m[s["name"]] = tables[s["name"]]
        in_maps.append(m)

    nc = _build_graph(plan, nt_total, 0)
    res = run_bass_kernel_spmd(nc, in_maps, core_ids=list(range(NCORES)))
    LAST_RESULT = res

    # --- unshard: undo the sort permutation, widen to f32 ---
    full = np.empty((n_tok, D_PROJ), np.float32)
    for c in range(NCORES):
        rows_by = {
            name: np.asarray(res.results[c][name])
            .transpose(1, 0, 2)
            .reshape(-1, D_PROJ)
            for name in ("outmm", "outpp")
        }
        for (name, tb, per_core_toks) in unshard:
            toks = per_core_toks[c]
            if len(toks):
                full[toks] = rows_by[name][tb * 128 : tb * 128 + len(toks)]
    B, S = np.asarray(inp).shape
    return full.reshape(B, S, D_PROJ)
